# revision 3
# baseline (speedup 1.0000x reference)
"""Trainium2 kernel for the damped-spring (DMP-style) batched scan.

Reference semantics (per batch b, dof n, with x0=dx0=0):
    ddx_t = ax*(bx*(goal - x_t) - dx_t) + f_t
    dx    += ddx_t * DT
    x     += dx * DT
    traj[..., t] = x

This is a linear time-invariant 2nd-order recurrence in s=(x,dx):
    s_{t+1} = A s_t + v * u_t,   u_t = f_t + ax*bx*goal,  v = (DT^2, DT)
whose x-transfer function is  DT^2 * z / ((z-lam1)(z-lam2)).  With real
eigenvalues lam1, lam2 the whole scan factors into a CASCADE of two
first-order scans, which map 1:1 onto the vector engine's
TensorTensorScanArith instruction (state = data0*state + data1):
    y1_t = lam1*y1_{t-1} + (DT^2*f_t + ax*bx*DT^2*goal)
    y2_t = lam2*y2_{t-1} + y1_t
    traj_t = y2_t

Sharding: data-parallel over batch across 8 cores; 2048*16/8 = 4096
independent sequences per core, processed as 32 tiles of (128 part x
4096 time).
"""

import os
import numpy as np

_B, _N, _T = 2048, 16, 4096
_NCORES = 8
_P = 128
_SEQ = (_B // _NCORES) * _N          # 4096 sequences per core
_NTILES = _SEQ // _P                 # 32
_DT = float(np.float32(0.01))

# Stash of the last BassKernelResults (exec_time_ns etc.) for test harnesses.
LAST_RESULT = None


def _eigs(ax: float, bx: float):
    """Eigenvalues of the 2x2 step matrix; None if complex (not expected)."""
    a, b, dt = float(ax), float(bx), _DT
    A00 = 1.0 - a * b * dt * dt
    A01 = dt * (1.0 - a * dt)
    A10 = -a * b * dt
    A11 = 1.0 - a * dt
    tr = A00 + A11
    det = A00 * A11 - A01 * A10
    disc = tr * tr - 4.0 * det
    if disc < 0.0:
        return None
    s = disc ** 0.5
    return (tr + s) / 2.0, (tr - s) / 2.0


def _kernel_numpy(force, goal, ax, bx):
    """Exact fallback (never expected on the real problem; complex poles)."""
    B, N, T = force.shape
    dt = np.float32(_DT)
    x = np.zeros((B, N), np.float32)
    dx = np.zeros((B, N), np.float32)
    out = np.empty((B, N, T), np.float32)
    axf, bxf = np.float32(ax), np.float32(bx)
    for t in range(T):
        ddx = axf * (bxf * (goal - x) - dx) + force[:, :, t]
        dx = dx + ddx * dt
        x = x + dx * dt
        out[:, :, t] = x
    return out


def _build_program(lam1: float, lam2: float, scale: float,
                   seq: int = _SEQ, t: int = _T):
    import concourse.bacc as bacc
    import concourse.mybir as mybir
    from concourse.tile import TileContext

    f32 = mybir.dt.float32
    ntiles = seq // _P
    # Bacc (not raw Bass): its compile() runs generate_event_semaphores,
    # which legalizes the >1-sync-wait-per-instruction cases Tile emits.
    nc = bacc.Bacc()
    force_d = nc.declare_dram_parameter("force", [seq, t], f32, isOutput=False)
    bias_d = nc.declare_dram_parameter("bias", [_P, ntiles], f32, isOutput=False)
    out_d = nc.declare_dram_parameter("out", [seq, t], f32, isOutput=True)

    with TileContext(nc) as tc:
        with tc.tile_pool(name="const", bufs=1) as cpool, \
             tc.tile_pool(name="work", bufs=2) as pool:
            lam1_t = cpool.tile([_P, t], f32, tag="lam1")
            lam2_t = cpool.tile([_P, t], f32, tag="lam2")
            nc.vector.memset(lam1_t[:], lam1)
            nc.vector.memset(lam2_t[:], lam2)
            bias_t = cpool.tile([_P, ntiles], f32, tag="bias")
            nc.sync.dma_start(out=bias_t[:], in_=bias_d[:, :])
            for i in range(ntiles):
                rows = slice(i * _P, (i + 1) * _P)
                f = pool.tile([_P, t], f32, tag="f")
                nc.sync.dma_start(out=f[:], in_=force_d[rows, :])
                u = pool.tile([_P, t], f32, tag="u")
                nc.scalar.activation(
                    u[:], f[:], mybir.ActivationFunctionType.Identity,
                    bias=bias_t[:, i:i + 1], scale=scale,
                )
                y1 = pool.tile([_P, t], f32, tag="y1")
                nc.vector.tensor_tensor_scan(
                    y1[:], lam1_t[:], u[:], 0.0,
                    mybir.AluOpType.mult, mybir.AluOpType.add,
                )
                y2 = pool.tile([_P, t], f32, tag="y2")
                nc.vector.tensor_tensor_scan(
                    y2[:], lam2_t[:], y1[:], 0.0,
                    mybir.AluOpType.mult, mybir.AluOpType.add,
                )
                nc.sync.dma_start(out=out_d[rows, :], in_=y2[:])
    nc.compile()
    return nc


def kernel(force, goal, ax, bx):
    global LAST_RESULT
    force = np.ascontiguousarray(np.asarray(force, dtype=np.float32))
    goal = np.ascontiguousarray(np.asarray(goal, dtype=np.float32))
    assert force.shape == (_B, _N, _T), force.shape

    lams = _eigs(float(ax), float(bx))
    if lams is None:
        return _kernel_numpy(force, goal, ax, bx)
    lam1, lam2 = lams
    scale = _DT * _DT

    from concourse.bass_utils import run_bass_kernel_spmd

    nc = _build_program(lam1, lam2, scale)

    # bias per sequence: ax*bx*DT^2*goal, laid out (P, NTILES) per core so a
    # single column is the per-partition activation bias for tile i.
    bias_all = (np.float32(float(ax) * float(bx)) * goal *
                np.float32(scale)).astype(np.float32)          # (B, N)
    bias_all = bias_all.reshape(_NCORES, _NTILES, _P)          # seq = i*P + p
    force_sh = force.reshape(_NCORES, _SEQ, _T)

    in_maps = [
        {
            "force": force_sh[c],
            "bias": np.ascontiguousarray(bias_all[c].T),       # (P, NTILES)
        }
        for c in range(_NCORES)
    ]
    res = run_bass_kernel_spmd(
        nc, in_maps, list(range(_NCORES)),
        trace=bool(os.environ.get("KERNEL_TRACE")),
    )
    LAST_RESULT = res
    out = np.stack([res.results[c]["out"] for c in range(_NCORES)])
    return out.reshape(_B, _N, _T)


# revision 6
# speedup vs baseline: 1.0108x; 1.0108x over previous
"""Trainium2 kernel for the damped-spring (DMP-style) batched scan.

Reference semantics (per batch b, dof n, x0=dx0=0):
    ddx_t = ax*(bx*(goal - x_t) - dx_t) + f_t
    dx += ddx_t*DT;  x += dx*DT;  traj[..., t] = x

This is a linear time-invariant 2nd-order recurrence in s=(x,dx):
    s_{t+1} = A s_t + v*u_t,   u_t = f_t + ax*bx*goal,  v = (DT^2, DT)
whose x-transfer function is  DT^2 * z / ((z-lam1)(z-lam2)).  With real
eigenvalues lam1/lam2 the scan factors into a CASCADE of two first-order
scans, mapping 1:1 onto the vector engine's TensorTensorScanArith
(state = data0*state + data1, fp32 internal state):
    y1_t = lam1*y1_{t-1} + u'_t        u'_t = scale*(f_t + ax*bx*goal)
    y2_t = lam2*y2_{t-1} + y1_t        traj_t = y2_t   (scale = DT^2*corr)

Two device implementations:
  * bf16 (default): 16-bit TTS runs 2 elem-cycles -> 1 on DVE and halves
    input DMA.  lam quantization to bf16 is compensated by a DC-gain
    correction `corr`; a host-side impulse-response check guards that the
    quantized filter is accurate enough, else falls back to fp32.
  * fp32: exact (rel err ~2e-7), ~1.7x slower (DVE scan-bound).

Sharding: data-parallel over batch across 8 cores; 2048*16/8 = 4096
sequences per core = 32 tiles of (128 partitions x 4096 time).
"""

import os
import numpy as np

_B, _N, _T = 2048, 16, 4096
_NCORES = 8
_P = 128
_SEQ = (_B // _NCORES) * _N          # 4096 sequences per core
_NTILES = _SEQ // _P                 # 32
_DT = float(np.float32(0.01))

LAST_RESULT = None                   # BassKernelResults stash for harnesses


def _eigs(ax: float, bx: float):
    a, b, dt = float(ax), float(bx), _DT
    A00 = 1.0 - a * b * dt * dt
    A01 = dt * (1.0 - a * dt)
    A10 = -a * b * dt
    A11 = 1.0 - a * dt
    tr = A00 + A11
    det = A00 * A11 - A01 * A10
    disc = tr * tr - 4.0 * det
    if disc <= 0.0:
        return None
    s = disc ** 0.5
    return (tr + s) / 2.0, (tr - s) / 2.0


def _bf16(x: float) -> float:
    """Round a python float to the nearest bfloat16 value."""
    v = np.float32(x).view(np.uint32)
    rounded = (int(v) + 0x8000) & 0xFFFF0000
    return float(np.uint32(rounded).view(np.float32))


def _bf16_filter_ok(lam1, lam2, l1b, l2b, corr, n=512, tol=5e-3):
    """Compare DC-corrected quantized impulse response vs the exact one."""
    k = np.arange(1, n + 1, dtype=np.float64)
    h = (lam1 ** k - lam2 ** k) / (lam1 - lam2)
    if abs(l1b - l2b) < 1e-12:
        hq = k * (l1b ** (k - 1))
    else:
        hq = (l1b ** k - l2b ** k) / (l1b - l2b)
    hq = hq * corr
    return np.linalg.norm(hq - h) / np.linalg.norm(h) < tol


def _kernel_numpy(force, goal, ax, bx):
    """Exact fallback (complex poles; not expected for this problem)."""
    B, N, T = force.shape
    dt = np.float32(_DT)
    x = np.zeros((B, N), np.float32)
    dx = np.zeros((B, N), np.float32)
    out = np.empty((B, N, T), np.float32)
    axf, bxf = np.float32(ax), np.float32(bx)
    for t in range(T):
        ddx = axf * (bxf * (goal - x) - dx) + force[:, :, t]
        dx = dx + ddx * dt
        x = x + dx * dt
        out[:, :, t] = x
    return out


def _build_program(lam1: float, lam2: float, scale: float,
                   seq: int = _SEQ, t: int = _T, dtype: str = "bf16"):
    import concourse.bacc as bacc
    import concourse.mybir as mybir
    from concourse.tile import TileContext

    f32 = mybir.dt.float32
    dat = mybir.dt.bfloat16 if dtype == "bf16" else f32
    MULT, ADD = mybir.AluOpType.mult, mybir.AluOpType.add
    ident = mybir.ActivationFunctionType.Identity
    ntiles = seq // _P
    # Bacc (not raw Bass): its compile() runs generate_event_semaphores,
    # which legalizes the >1-sync-wait-per-instruction cases Tile emits.
    nc = bacc.Bacc()
    force_d = nc.declare_dram_parameter("force", [seq, t], dat, isOutput=False)
    bias_d = nc.declare_dram_parameter("bias", [_P, ntiles], f32, isOutput=False)
    out_d = nc.declare_dram_parameter("out", [seq, t], f32, isOutput=True)

    with TileContext(nc) as tc:
        with tc.tile_pool(name="const", bufs=1) as cpool, \
             tc.tile_pool(name="io", bufs=3) as iop, \
             tc.tile_pool(name="work", bufs=2) as pool:
            lam1_t = cpool.tile([_P, t], dat, tag="lam1")
            lam2_t = cpool.tile([_P, t], dat, tag="lam2")
            nc.vector.memset(lam1_t[:], lam1)
            nc.vector.memset(lam2_t[:], lam2)
            bias_t = cpool.tile([_P, ntiles], f32, tag="bias")
            nc.sync.dma_start(out=bias_t[:], in_=bias_d[:, :])
            for i in range(ntiles):
                rows = slice(i * _P, (i + 1) * _P)
                f = iop.tile([_P, t], dat, tag="f")
                nc.sync.dma_start(out=f[:], in_=force_d[rows, :])
                u = pool.tile([_P, t], dat, tag="u")
                nc.scalar.activation(u[:], f[:], ident,
                                     bias=bias_t[:, i:i + 1], scale=scale)
                y1 = pool.tile([_P, t], dat, tag="y1")
                nc.vector.tensor_tensor_scan(y1[:], lam1_t[:], u[:], 0.0,
                                             MULT, ADD)
                y2 = pool.tile([_P, t], dat, tag="y2")
                nc.vector.tensor_tensor_scan(y2[:], lam2_t[:], y1[:], 0.0,
                                             MULT, ADD)
                if dtype == "bf16":
                    tr_t = iop.tile([_P, t], f32, tag="tr")
                    nc.scalar.activation(tr_t[:], y2[:], ident,
                                         bias=0.0, scale=1.0)
                else:
                    tr_t = y2
                nc.sync.dma_start(out=out_d[rows, :], in_=tr_t[:])
    nc.compile()
    return nc


def kernel(force, goal, ax, bx):
    global LAST_RESULT
    force = np.ascontiguousarray(np.asarray(force, dtype=np.float32))
    goal = np.ascontiguousarray(np.asarray(goal, dtype=np.float32))
    assert force.shape == (_B, _N, _T), force.shape

    lams = _eigs(float(ax), float(bx))
    if lams is None:
        return _kernel_numpy(force, goal, ax, bx)
    lam1, lam2 = lams

    impl = os.environ.get("KERNEL_IMPL", "bf16")
    if impl == "bf16":
        l1b, l2b = _bf16(lam1), _bf16(lam2)
        corr = ((1 - l1b) * (1 - l2b)) / ((1 - lam1) * (1 - lam2))
        if not _bf16_filter_ok(lam1, lam2, l1b, l2b, corr):
            impl = "fp32"

    from concourse.bass_utils import run_bass_kernel_spmd

    if impl == "bf16":
        scale = _DT * _DT * corr
        nc = _build_program(l1b, l2b, scale, dtype="bf16")
        import ml_dtypes
        force_sh = force.reshape(_NCORES, _SEQ, _T).astype(ml_dtypes.bfloat16)
    else:
        scale = _DT * _DT
        nc = _build_program(lam1, lam2, scale, dtype="fp32")
        force_sh = force.reshape(_NCORES, _SEQ, _T)

    # per-sequence bias scale*ax*bx*goal, laid out (P, NTILES) per core so
    # one column is the per-partition activation bias for tile i.
    bias_all = (np.float32(float(ax) * float(bx)) * goal *
                np.float32(scale)).astype(np.float32)          # (B, N)
    bias_all = bias_all.reshape(_NCORES, _NTILES, _P)          # seq = i*P + p

    in_maps = [
        {
            "force": force_sh[c],
            "bias": np.ascontiguousarray(bias_all[c].T),       # (P, NTILES)
        }
        for c in range(_NCORES)
    ]
    res = run_bass_kernel_spmd(
        nc, in_maps, list(range(_NCORES)),
        trace=bool(os.environ.get("KERNEL_TRACE")),
    )
    LAST_RESULT = res
    out = np.stack([res.results[c]["out"] for c in range(_NCORES)])
    return out.reshape(_B, _N, _T)


# revision 8
# speedup vs baseline: 1.0269x; 1.0159x over previous
"""Trainium2 kernel for the damped-spring (DMP-style) batched scan.

Reference semantics (per batch b, dof n, x0=dx0=0):
    ddx_t = ax*(bx*(goal - x_t) - dx_t) + f_t
    dx += ddx_t*DT;  x += dx*DT;  traj[..., t] = x

This is a linear time-invariant 2nd-order recurrence in s=(x,dx):
    s_{t+1} = A s_t + v*u_t,   u_t = f_t + ax*bx*goal,  v = (DT^2, DT)
whose x-transfer function is  DT^2 * z / ((z-lam1)(z-lam2)).  With real
eigenvalues lam1/lam2 the scan factors into a CASCADE of two first-order
scans, mapping 1:1 onto the vector engine's TensorTensorScanArith
(state = data0*state + data1, fp32 internal state):
    y1_t = lam1*y1_{t-1} + u'_t        u'_t = scale*(f_t + ax*bx*goal)
    y2_t = lam2*y2_{t-1} + y1_t        traj_t = y2_t   (scale = DT^2*corr)

Two device implementations:
  * bf16 (default): 16-bit TTS runs 2 elem-cycles -> 1 on DVE and halves
    input DMA.  lam quantization to bf16 is compensated by a DC-gain
    correction `corr`; a host-side impulse-response check guards that the
    quantized filter is accurate enough, else falls back to fp32.
  * fp32: exact (rel err ~2e-7), ~1.7x slower (DVE scan-bound).

Sharding: data-parallel over batch across 8 cores; 2048*16/8 = 4096
sequences per core = 32 tiles of (128 partitions x 4096 time).
"""

import os
import numpy as np

_B, _N, _T = 2048, 16, 4096
_NCORES = 8
_P = 128
_SEQ = (_B // _NCORES) * _N          # 4096 sequences per core
_NTILES = _SEQ // _P                 # 32
_DT = float(np.float32(0.01))

LAST_RESULT = None                   # BassKernelResults stash for harnesses


def _eigs(ax: float, bx: float):
    a, b, dt = float(ax), float(bx), _DT
    A00 = 1.0 - a * b * dt * dt
    A01 = dt * (1.0 - a * dt)
    A10 = -a * b * dt
    A11 = 1.0 - a * dt
    tr = A00 + A11
    det = A00 * A11 - A01 * A10
    disc = tr * tr - 4.0 * det
    if disc <= 0.0:
        return None
    s = disc ** 0.5
    return (tr + s) / 2.0, (tr - s) / 2.0


def _bf16(x: float) -> float:
    """Round a python float to the nearest bfloat16 value."""
    v = np.float32(x).view(np.uint32)
    rounded = (int(v) + 0x8000) & 0xFFFF0000
    return float(np.uint32(rounded).view(np.float32))


def _bf16_ok_on_sample(force, goal, ax, bx, l1b, l2b, scale, tol=8e-3):
    """End-to-end error of the bf16 pipeline on a sample of the real data.

    Runs the exact fp32 reference recurrence and the bf16-emulated cascade
    (bf16 data / fp32 scan state, matching HW TensorTensorScan semantics)
    on ~32 sequences and accepts bf16 iff the relative L2 error < tol.
    """
    import ml_dtypes
    bf = ml_dtypes.bfloat16
    f = force[:: max(1, force.shape[0] // 32), 0, :].astype(np.float32)
    g = goal[:: max(1, goal.shape[0] // 32), 0].astype(np.float32)
    S, T = f.shape
    dt = np.float32(_DT)
    axf, bxf = np.float32(ax), np.float32(bx)
    x = np.zeros(S, np.float32)
    dx = np.zeros(S, np.float32)
    ref = np.empty((S, T), np.float32)
    for t in range(T):
        ddx = axf * (bxf * (g - x) - dx) + f[:, t]
        dx = dx + ddx * dt
        x = x + dx * dt
        ref[:, t] = x
    fb = f.astype(bf).astype(np.float32)
    u = (fb * np.float32(scale) +
         (axf * bxf * g * np.float32(scale))[:, None]).astype(bf)
    s1 = np.zeros(S, np.float32)
    s2 = np.zeros(S, np.float32)
    out = np.empty((S, T), np.float32)
    l1f, l2f = np.float32(l1b), np.float32(l2b)
    for t in range(T):
        s1 = l1f * s1 + u[:, t].astype(np.float32)
        s2 = l2f * s2 + np.float32(s1.astype(bf))
        out[:, t] = s2.astype(bf).astype(np.float32)
    rel = (np.linalg.norm((out - ref).ravel().astype(np.float64)) /
           np.linalg.norm(ref.ravel().astype(np.float64)))
    return rel < tol


def _kernel_numpy(force, goal, ax, bx):
    """Exact fallback (complex poles; not expected for this problem)."""
    B, N, T = force.shape
    dt = np.float32(_DT)
    x = np.zeros((B, N), np.float32)
    dx = np.zeros((B, N), np.float32)
    out = np.empty((B, N, T), np.float32)
    axf, bxf = np.float32(ax), np.float32(bx)
    for t in range(T):
        ddx = axf * (bxf * (goal - x) - dx) + force[:, :, t]
        dx = dx + ddx * dt
        x = x + dx * dt
        out[:, :, t] = x
    return out


def _build_program(lam1: float, lam2: float, scale: float,
                   seq: int = _SEQ, t: int = _T, dtype: str = "bf16"):
    import concourse.bacc as bacc
    import concourse.mybir as mybir
    from concourse.tile import TileContext

    f32 = mybir.dt.float32
    dat = mybir.dt.bfloat16 if dtype == "bf16" else f32
    MULT, ADD = mybir.AluOpType.mult, mybir.AluOpType.add
    ident = mybir.ActivationFunctionType.Identity
    ntiles = seq // _P
    # Bacc (not raw Bass): its compile() runs generate_event_semaphores,
    # which legalizes the >1-sync-wait-per-instruction cases Tile emits.
    nc = bacc.Bacc()
    force_d = nc.declare_dram_parameter("force", [seq, t], dat, isOutput=False)
    bias_d = nc.declare_dram_parameter("bias", [_P, ntiles], f32, isOutput=False)
    out_d = nc.declare_dram_parameter("out", [seq, t], f32, isOutput=True)

    with TileContext(nc) as tc:
        with tc.tile_pool(name="const", bufs=1) as cpool, \
             tc.tile_pool(name="io", bufs=3) as iop, \
             tc.tile_pool(name="work", bufs=2) as pool:
            lam1_t = cpool.tile([_P, t], dat, tag="lam1")
            lam2_t = cpool.tile([_P, t], dat, tag="lam2")
            nc.vector.memset(lam1_t[:], lam1)
            nc.vector.memset(lam2_t[:], lam2)
            bias_t = cpool.tile([_P, ntiles], f32, tag="bias")
            nc.sync.dma_start(out=bias_t[:], in_=bias_d[:, :])
            for i in range(ntiles):
                rows = slice(i * _P, (i + 1) * _P)
                f = iop.tile([_P, t], dat, tag="f")
                nc.sync.dma_start(out=f[:], in_=force_d[rows, :])
                u = pool.tile([_P, t], dat, tag="u")
                nc.scalar.activation(u[:], f[:], ident,
                                     bias=bias_t[:, i:i + 1], scale=scale)
                y1 = pool.tile([_P, t], dat, tag="y1")
                nc.vector.tensor_tensor_scan(y1[:], lam1_t[:], u[:], 0.0,
                                             MULT, ADD)
                y2 = pool.tile([_P, t], dat, tag="y2")
                nc.vector.tensor_tensor_scan(y2[:], lam2_t[:], y1[:], 0.0,
                                             MULT, ADD)
                if dtype == "bf16":
                    tr_t = iop.tile([_P, t], f32, tag="tr")
                    nc.scalar.activation(tr_t[:], y2[:], ident,
                                         bias=0.0, scale=1.0)
                else:
                    tr_t = y2
                nc.sync.dma_start(out=out_d[rows, :], in_=tr_t[:])
    nc.compile()
    return nc


def kernel(force, goal, ax, bx):
    global LAST_RESULT
    force = np.ascontiguousarray(np.asarray(force, dtype=np.float32))
    goal = np.ascontiguousarray(np.asarray(goal, dtype=np.float32))
    assert force.shape == (_B, _N, _T), force.shape

    lams = _eigs(float(ax), float(bx))
    if lams is None:
        return _kernel_numpy(force, goal, ax, bx)
    lam1, lam2 = lams

    impl = os.environ.get("KERNEL_IMPL", "bf16")
    if impl == "bf16":
        l1b, l2b = _bf16(lam1), _bf16(lam2)
        corr = ((1 - l1b) * (1 - l2b)) / ((1 - lam1) * (1 - lam2))
        if not _bf16_ok_on_sample(force, goal, ax, bx, l1b, l2b,
                                  _DT * _DT * corr):
            impl = "fp32"

    from concourse.bass_utils import run_bass_kernel_spmd

    if impl == "bf16":
        scale = _DT * _DT * corr
        nc = _build_program(l1b, l2b, scale, dtype="bf16")
        import ml_dtypes
        force_sh = force.reshape(_NCORES, _SEQ, _T).astype(ml_dtypes.bfloat16)
    else:
        scale = _DT * _DT
        nc = _build_program(lam1, lam2, scale, dtype="fp32")
        force_sh = force.reshape(_NCORES, _SEQ, _T)

    # per-sequence bias scale*ax*bx*goal, laid out (P, NTILES) per core so
    # one column is the per-partition activation bias for tile i.
    bias_all = (np.float32(float(ax) * float(bx)) * goal *
                np.float32(scale)).astype(np.float32)          # (B, N)
    bias_all = bias_all.reshape(_NCORES, _NTILES, _P)          # seq = i*P + p

    in_maps = [
        {
            "force": force_sh[c],
            "bias": np.ascontiguousarray(bias_all[c].T),       # (P, NTILES)
        }
        for c in range(_NCORES)
    ]
    res = run_bass_kernel_spmd(
        nc, in_maps, list(range(_NCORES)),
        trace=bool(os.environ.get("KERNEL_TRACE")),
    )
    LAST_RESULT = res
    out = np.stack([res.results[c]["out"] for c in range(_NCORES)])
    return out.reshape(_B, _N, _T)
